# revision 17
# baseline (speedup 1.0000x reference)
"""Fused additive-attention kernel for Trainium2 (8 NeuronCores, SPMD).

Computes  w = softmax_K( mask ? (Wl . tanh(vW_v^T + qW_q^T) + bl) : -1e9 )
without materializing the [B,N,S,K,H] joint_repr intermediate.

Key ideas over the naive formulation:
  * Masked boxes get weight exactly 0 (exp(-1e9) underflows), so only the
    unmasked boxes are computed.  Host gathers each batch's unmasked box
    list; batches are paired onto cores large-with-small so the padded
    per-slot counts (K0 for the core's first batch, K1 for its second) stay
    near the true max.  Masked/padding slots are -1e9'd on device and the
    host scatters results back into the full [B,N,S,K] output (zeros for
    masked boxes).
  * The broadcast add vp[b,k,h] + qp[b,n,s,h] runs as DVE tensor_scalar_add
    with a per-partition [128,1] vp operand (high DVE perf mode).
  * All device tensors are packed on host into their SBUF layout
    ([128, ...] partition-major, contiguous per partition) so every DMA is
    large-descriptor and the single queue is bandwidth- not
    descriptor-rate-bound.  Weights are split per h-chunk so compute starts
    after ~1/3 of the bytes.
  * Biases bq/bv are folded into QPs/VP at projection time; bl cancels in
    softmax.  Logits are bounded (|logit| <= sum|Wl|), so softmax skips the
    max-subtraction pass.

Per-core dataflow (h on partitions for the hot loop), phased per h-chunk:
  QPs[hc] [128(h), 512(b,ns)] = Wq-slice.T @ qT + bq   (PE psum, DVE copy)
  VP[hc]  [128(h), S(slots)]  = Wv-slice.T @ vG + bv   (S = K0+K1)
  JT      [128, strip(kk)*256] bf16 = QPs-half + VP[slot]  (DVE)
  tanh in-place on JT (one ACT op per slot-group)
  logits  psum: batch b uses cols b*256:(b+1)*256 and PE col-strips
          (0, 32) for b0 / (64, 96) for b1 (disjoint psum partition rows:
          start=True zeroes the whole 2KB bank row), accumulated over hc
          with zero-padded Wl lhsT (pair j, j+P shares lhsT; tile_position).
  softmax over slots after PE-transposing logits to [ns, slots];
  hc3 runs b1 before b0 so b1's softmax hides under b0's tanh stream.
"""

import os
import sys

import numpy as np

sys.path.insert(0, "/opt/trn_rl_repo")

import concourse.bass as bass
import concourse.mybir as mybir
from concourse import bacc, bass_utils
from concourse.tile import TileContext

# Problem shapes (hardcoded per contract -- kernel.py must be self-contained)
B, N, S, K = 16, 4, 64, 50
VD, QD, H = 1024, 768, 512
NCORES = 8
BPC = B // NCORES          # batches per core = 2
NS = BPC * N * S           # 512 rows (b, n, s) per core
NSB = NS // BPC            # 256 rows per batch
HC = H // 128              # 4 h-chunks
QC = QD // 128             # 6 qd-chunks
VC = VD // 128             # 8 vd-chunks

F32 = mybir.dt.float32
BF16 = mybir.dt.bfloat16

_CACHE = {}


def _groups(hc, b, P):
    """Pair-index groups for (hc, batch).  First groups of the whole kernel
    are small so the first tanh issues as early as possible; afterwards one
    big group per (hc, b) amortizes the ACT per-op bubble."""
    pairs = list(range(P))
    if hc == 0 and b == 0:
        gs = [pairs[0:2], pairs[2:9], pairs[9:]]
    elif hc == 0:
        h = (P + 1) // 2
        gs = [pairs[:h], pairs[h:]]
    else:
        gs = [pairs]
    return [g for g in gs if g]


def _build_nc(K0, K1):
    P0, P1 = K0 // 2, K1 // 2
    SL = K0 + K1               # slots per core
    KMAX = max(K0, K1)
    WZ0 = P0 * P0              # wlz cols per hc for batch 0
    WZC = WZ0 + P1 * P1        # wlz cols per hc total

    nc = bacc.Bacc("TRN2", target_bir_lowering=False)

    # All inputs are pre-packed on host into SBUF layout [128, ...]
    qT_h = nc.dram_tensor("qT", [128, QC * NS], BF16, kind="ExternalInput")
    vG_h = nc.dram_tensor("vG", [128, VC * SL], BF16, kind="ExternalInput")
    wq_h = [
        nc.dram_tensor(f"wq{hc}", [128, QC * 128], BF16, kind="ExternalInput")
        for hc in range(HC)
    ]
    wv_h = [
        nc.dram_tensor(f"wv{hc}", [128, VC * 128], BF16, kind="ExternalInput")
        for hc in range(HC)
    ]
    # packed [128, 12]: cols 0:4 Wl chunks, 4:8 bq chunks, 8:12 bv chunks
    wlb_h = nc.dram_tensor("wlb", [128, 12], F32, kind="ExternalInput")
    # zero-padded Wl variants, per (hc, b, j): [128, Pb] slab, col c = Wl*(c==j)
    wlz_h = nc.dram_tensor("wlz", [128, HC * WZC], BF16, kind="ExternalInput")
    # additive mask: col s = 0.0 for a real slot, -1e9 for padding/masked
    msk_h = nc.dram_tensor("msk", [128, SL], F32, kind="ExternalInput")
    id_h = nc.dram_tensor("ident", [128, 128], F32, kind="ExternalInput")
    # out col (nsc, j): w[ns = nsc*128 + p, slot j]
    out_h = nc.dram_tensor(
        "out", [128, (NS // 128) * KMAX], F32, kind="ExternalOutput"
    )

    with TileContext(nc) as tc:
        with (
            tc.tile_pool(name="persist", bufs=1) as pp,
            tc.tile_pool(name="ppsum", bufs=1, space="PSUM") as ppsum,
            tc.tile_pool(name="projps", bufs=2, space="PSUM") as pjps,
            tc.tile_pool(name="smpsum", bufs=2, space="PSUM") as sps,
        ):
            # ---- DMA loads.  qT is packed batch-half-major ([128, bh, c, j])
            # so the b0 projection chain starts after ~1MB of DMA ----
            wlb = pp.tile_from(wlb_h[:, :], name="wlb")
            qts = pp.tile([128, 2, QC, NSB], BF16, name="qts")
            qts_f = qts[:, :, :, :].rearrange("p h c j -> p (h c j)")
            HB = QC * NSB
            wqt = [None] * HC
            wvt = [None] * HC
            vts = pp.tile_from(vG_h[:, :], name="vts")
            wvt[0] = pp.tile_from(wv_h[0][:, :], name="wv0")
            nc.sync.dma_start(qts_f[:, 0:HB], qT_h[:, 0:HB])
            wqt[0] = pp.tile_from(wq_h[0][:, :], name="wq0")
            nc.sync.dma_start(qts_f[:, HB : 2 * HB], qT_h[:, HB : 2 * HB])
            wqt[1] = pp.tile_from(wq_h[1][:, :], name="wq1")
            wvt[1] = pp.tile_from(wv_h[1][:, :], name="wv1")
            wlz = pp.tile_from(wlz_h[:, :], name="wlz")
            msk = pp.tile_from(msk_h[:, :], name="msk")
            ident = pp.tile_from(id_h[:, :], name="ident")
            wqt[2] = pp.tile_from(wq_h[2][:, :], name="wq2")
            wvt[2] = pp.tile_from(wv_h[2][:, :], name="wv2")
            wqt[3] = pp.tile_from(wq_h[3][:, :], name="wq3")
            wvt[3] = pp.tile_from(wv_h[3][:, :], name="wv3")

            # qp (all h-chunks): [128, (hc, b, ns)] bf16, +bq folded
            QPs = pp.tile([128, HC * NS], BF16, name="QPs")
            # vp slot table: [128, (hc, slot)] f32, +bv folded
            VP = pp.tile([128, HC * SL], F32, name="VP")

            # logits psum: batch b owns cols b*256:(b+1)*256 and PE col-strips
            # (0, 32) for b0 / (64, 96) for b1 -> psum rows 0:P0, 32:32+P0,
            # 64:64+P1, 96:96+P1.  Strips of the two batches must not share
            # psum partition rows: start=True zeroes the whole 2KB bank row.
            ps_log = ppsum.tile([128, NS], F32, name="ps_log")

            def proj_phase(hc):
                """Compute QPs/VP h-chunk hc.  Projections, then the bias-fold
                copies, run per batch-half so b0's QPs is ready before b1's
                qT half has even arrived (hc0 startup)."""
                pv = pjps.tile([128, SL], F32, tag="pv", name="pv")
                for vc in range(VC):
                    nc.tensor.matmul(
                        pv[:, :],
                        wvt[hc][:, vc * 128 : (vc + 1) * 128],
                        vts[:, vc * SL : (vc + 1) * SL],
                        start=(vc == 0),
                        stop=(vc == VC - 1),
                    )
                nc.vector.tensor_scalar_add(
                    VP[:, hc * SL : (hc + 1) * SL],
                    pv[:, :],
                    wlb[:, 2 * HC + hc : 2 * HC + hc + 1],
                )
                pq = pjps.tile([128, NS], F32, tag="pq", name="pq")
                for bh in range(2):
                    for qc in range(QC):
                        nc.tensor.matmul(
                            pq[:, bh * NSB : (bh + 1) * NSB],
                            wqt[hc][:, qc * 128 : (qc + 1) * 128],
                            qts[:, bh, qc, :],
                            start=(qc == 0),
                            stop=(qc == QC - 1),
                        )
                    nc.vector.tensor_scalar_add(
                        QPs[:, hc * NS + bh * NSB : hc * NS + (bh + 1) * NSB],
                        pq[:, bh * NSB : (bh + 1) * NSB],
                        wlb[:, HC + hc : HC + hc + 1],
                    )

            def main_hc(hc, b, mp, mid_cb=None):
                """Joint tanh + logit matmuls for one (h-chunk, batch)."""
                P = P0 if b == 0 else P1
                wzb = hc * WZC + (0 if b == 0 else WZ0)
                vcb = hc * SL + b * K0
                qpo = hc * NS + b * NSB
                groups = _groups(hc, b, P)
                mid_g = min(1, len(groups) - 1)
                for g, js in enumerate(groups):
                    if g == mid_g and mid_cb is not None:
                        mid_cb()
                    L = len(js)
                    JT = mp.tile([128, 2 * L * NSB], BF16, tag="JT", name="JT")
                    for kk in range(2 * L):
                        slot = js[kk] if kk < L else js[kk - L] + P
                        # route 1/3 of the broadcast adds to the otherwise
                        # idle GPSIMD engine to keep DVE off the critical path
                        eng = nc.gpsimd if kk % 3 == 2 else nc.vector
                        eng.tensor_scalar_add(
                            JT[:, kk * NSB : (kk + 1) * NSB],
                            QPs[:, qpo : qpo + NSB],
                            VP[:, vcb + slot : vcb + slot + 1],
                        )
                    # in-place tanh over the whole group
                    nc.scalar.activation(
                        JT[:, :], JT[:, :], mybir.ActivationFunctionType.Tanh
                    )
                    bcs = slice(b * NSB, (b + 1) * NSB)
                    r0 = 64 * b
                    r1 = r0 + 32
                    for jj, j in enumerate(js):
                        first = hc == 0 and g == 0 and jj == 0
                        last = hc == HC - 1 and g == len(groups) - 1 and jj == L - 1
                        nc.tensor.matmul(
                            ps_log[r0 : r0 + P, bcs],
                            wlz[:, wzb + j * P : wzb + (j + 1) * P],
                            JT[:, jj * NSB : (jj + 1) * NSB],
                            start=first,
                            stop=last,
                            tile_position=(0, r0),
                            skip_group_check=True,
                        )
                        nc.tensor.matmul(
                            ps_log[r1 : r1 + P, bcs],
                            wlz[:, wzb + j * P : wzb + (j + 1) * P],
                            JT[:, (L + jj) * NSB : (L + jj + 1) * NSB],
                            start=first,
                            stop=last,
                            tile_position=(0, r1),
                            skip_group_check=True,
                        )

            LGA = pp.tile([96 + 32, NSB], F32, name="LGA")
            W_all = pp.tile([128, NS // 128, KMAX], F32, name="W_all")

            def softmax_b(b):
                """Masked softmax for batch b (no max-pass: |logits| <~ 1.5)."""
                P = P0 if b == 0 else P1
                Kb = 2 * P
                r0 = 64 * b
                r1 = r0 + 32
                bcs = slice(b * NSB, (b + 1) * NSB)
                nc.vector.tensor_copy(LGA[r0 : r0 + P, :], ps_log[r0 : r0 + P, bcs])
                nc.vector.tensor_copy(LGA[r1 : r1 + P, :], ps_log[r1 : r1 + P, bcs])
                for nsb in range(NSB // 128):
                    nsc = b * 2 + nsb
                    ps_t = sps.tile([128, KMAX], F32, tag="ps_t", name="ps_t")
                    nc.tensor.transpose(
                        ps_t[:, 0:P],
                        LGA[r0 : r0 + P, nsb * 128 : (nsb + 1) * 128],
                        ident[r0 : r0 + P, r0 : r0 + P],
                        tile_position=(r0, 0),
                    )
                    nc.tensor.transpose(
                        ps_t[:, P : 2 * P],
                        LGA[r1 : r1 + P, nsb * 128 : (nsb + 1) * 128],
                        ident[r1 : r1 + P, r1 : r1 + P],
                        tile_position=(r1, 0),
                    )
                    LT = pp.tile([128, KMAX], F32, name=f"LT{nsc}")
                    nc.vector.tensor_add(
                        LT[:, 0:Kb], ps_t[:, 0:Kb], msk[:, b * K0 : b * K0 + Kb]
                    )
                    EX = pp.tile([128, KMAX], F32, name=f"EX{nsc}")
                    sm = pp.tile([128, 1], F32, name=f"sm{nsc}")
                    nc.scalar.activation(
                        EX[:, 0:Kb], LT[:, 0:Kb],
                        mybir.ActivationFunctionType.Exp,
                        accum_out=sm[:, 0:1],
                    )
                    rs = pp.tile([128, 1], F32, name=f"rs{nsc}")
                    nc.vector.reciprocal(rs[:, :], sm[:, :])
                    nc.vector.tensor_scalar_mul(
                        W_all[:, nsc, 0:Kb], EX[:, 0:Kb], rs[:, 0:1]
                    )
                    if Kb < KMAX:
                        nc.vector.memset(W_all[:, nsc, Kb:KMAX], 0.0)
                out_v = out_h[:, :].rearrange("p (c j) -> p c j", j=KMAX)
                nc.sync.dma_start(
                    out_v[:, 2 * b : 2 * b + 2, :], W_all[:, 2 * b : 2 * b + 2, :]
                )

            proj_phase(0)
            with tc.tile_pool(name="main", bufs=4) as mp:

                def prio(hc):
                    def cb():
                        with tc.high_priority():
                            proj_phase(hc)

                    return cb

                main_hc(0, 0, mp, mid_cb=prio(1))
                main_hc(0, 1, mp, mid_cb=prio(2))
                main_hc(1, 0, mp, mid_cb=prio(3))
                main_hc(1, 1, mp)
                main_hc(2, 0, mp)
                main_hc(2, 1, mp)
                main_hc(3, 1, mp)   # b1 first: its softmax hides under b0 tanh
                softmax_b(1)
                main_hc(3, 0, mp)
                softmax_b(0)

    nc.finalize()
    return nc


def _plan(box_mask):
    """Pair batches onto cores large-with-small; return per-core batch ids,
    per-batch unmasked box index lists, and padded slot counts (K0, K1)."""
    mask = np.asarray(box_mask) > 0
    counts = mask.sum(axis=1)
    order = np.argsort(-counts, kind="stable")
    batA = order[:NCORES]
    batB = order[NCORES:][::-1]
    kidx = [np.nonzero(mask[b])[0] for b in range(B)]

    def pad_even(n):
        n = max(int(n), 2)
        return n + (n & 1)

    K0 = pad_even(counts[batA].max())
    K1 = pad_even(counts[batB].max())
    return batA, batB, kidx, K0, K1


def _dev_pack(M):
    """[C*128, X] row-major -> [128, C*X] partition-major (SBUF layout)."""
    D, X = M.shape
    C = D // 128
    return np.ascontiguousarray(
        M.reshape(C, 128, X).transpose(1, 0, 2).reshape(128, C * X)
    )


def _prep_in_maps(v, q, box_mask, Wv, bv, Wq, bq, Wl, plan):
    """Host-side layout prep: gather unmasked boxes, shard over cores,
    pack into device layouts."""
    import ml_dtypes

    batA, batB, kidx, K0, K1 = plan
    P0, P1 = K0 // 2, K1 // 2
    SL = K0 + K1
    WZ0 = P0 * P0
    WZC = WZ0 + P1 * P1

    v = np.asarray(v, np.float32).reshape(B, K, VD)
    q = np.asarray(q, np.float32).reshape(B, N * S, QD)

    WqT = np.asarray(Wq, np.float32).T                                # [QD, H]
    WvT = np.asarray(Wv, np.float32).T                                # [VD, H]
    shared = {}
    for hc in range(HC):
        shared[f"wq{hc}"] = _dev_pack(
            np.ascontiguousarray(WqT[:, hc * 128 : (hc + 1) * 128])
        ).astype(ml_dtypes.bfloat16)
        shared[f"wv{hc}"] = _dev_pack(
            np.ascontiguousarray(WvT[:, hc * 128 : (hc + 1) * 128])
        ).astype(ml_dtypes.bfloat16)
    wlb = np.zeros((128, 12), np.float32)
    wl_chunks = np.asarray(Wl, np.float32).reshape(HC, 128).T         # [128, hc]
    wlb[:, 0:4] = wl_chunks
    wlb[:, 4:8] = np.asarray(bq, np.float32).reshape(HC, 128).T
    wlb[:, 8:12] = np.asarray(bv, np.float32).reshape(HC, 128).T
    shared["wlb"] = wlb
    # zero-padded Wl variants per (hc, b, j)
    wlz = np.zeros((128, HC, WZC), np.float32)
    for j in range(P0):
        wlz[:, :, j * P0 + j] = wl_chunks
    for j in range(P1):
        wlz[:, :, WZ0 + j * P1 + j] = wl_chunks
    shared["wlz"] = wlz.reshape(128, HC * WZC).astype(ml_dtypes.bfloat16)
    shared["ident"] = np.eye(128, dtype=np.float32)

    in_maps = []
    for c in range(NCORES):
        qc = np.stack([q[batA[c]], q[batB[c]]])                       # [2,NSB,QD]
        # [128, (bh, c, j)]: batch-half-major so b0's projections need only
        # the first half of the transfer
        qT = (
            qc.transpose(2, 0, 1)                                     # [QD,2,NSB]
            .reshape(QC, 128, 2, NSB)
            .transpose(1, 2, 0, 3)
            .reshape(128, 2 * QC * NSB)
        )
        vg = np.zeros((SL, VD), np.float32)
        moff = np.full((1, SL), -1e9, np.float32)
        for bi, Kb, off in [(batA[c], K0, 0), (batB[c], K1, K0)]:
            ks = kidx[bi]
            vg[off : off + len(ks)] = v[bi, ks]
            moff[0, off : off + len(ks)] = 0.0
        vG = np.ascontiguousarray(vg.T)                               # [VD, SL]
        msk = np.ascontiguousarray(np.broadcast_to(moff, (128, SL)))
        in_maps.append(
            {
                "qT": np.ascontiguousarray(qT).astype(ml_dtypes.bfloat16),
                "vG": _dev_pack(vG).astype(ml_dtypes.bfloat16),
                "msk": msk,
                **shared,
            }
        )
    return in_maps


def kernel(v, q, box_mask, tags_attention, Wv, bv, Wq, bq, Wl, bl):
    # bl shifts all unmasked logits uniformly -> cancels in softmax.
    # tags_attention is unused by the reference module.
    plan = _plan(box_mask)
    batA, batB, kidx, K0, K1 = plan
    KMAX = max(K0, K1)
    key = (K0, K1)
    if key not in _CACHE:
        _CACHE[key] = _build_nc(K0, K1)
    nc = _CACHE[key]
    in_maps = _prep_in_maps(v, q, box_mask, Wv, bv, Wq, bq, Wl, plan)
    res = bass_utils.run_bass_kernel_spmd(
        nc,
        in_maps,
        core_ids=list(range(NCORES)),
        trace=bool(os.environ.get("KERNEL_TRACE")),
        tmpdir=os.environ.get("KERNEL_TMPDIR"),
    )
    _CACHE["last_result"] = res
    out = np.zeros((B, N * S, K), np.float32)
    for c in range(NCORES):
        w = (
            res.results[c]["out"]
            .reshape(128, NS // 128, KMAX)
            .transpose(1, 0, 2)
            .reshape(NS, KMAX)
        )
        for bi, off in [(batA[c], 0), (batB[c], NSB)]:
            ks = kidx[bi]
            if len(ks) == 0:
                out[bi, :, :] = 1.0 / K
            else:
                out[bi, :, ks] = w[off : off + NSB, : len(ks)].T
    return out.reshape(B, N, S, K)


# revision 18
# speedup vs baseline: 4.2982x; 4.2982x over previous
"""Fused additive-attention kernel for Trainium2 (8 NeuronCores, SPMD).

Computes  w = softmax_K( mask ? (Wl . tanh(vW_v^T + qW_q^T) + bl) : -1e9 )
without materializing the [B,N,S,K,H] joint_repr intermediate.

Key ideas over the naive formulation:
  * Masked boxes get weight exactly 0 (exp(-1e9) underflows), so only the
    unmasked boxes are computed.  Host gathers each batch's unmasked box
    list; batches are paired onto cores large-with-small so the padded
    per-slot counts (K0 for the core's first batch, K1 for its second) stay
    near the true max.  Masked/padding slots are -1e9'd on device and the
    host scatters results back into the full [B,N,S,K] output (zeros for
    masked boxes).
  * The broadcast add vp[b,k,h] + qp[b,n,s,h] runs as DVE tensor_scalar_add
    with a per-partition [128,1] vp operand (high DVE perf mode).
  * All device tensors are packed on host into their SBUF layout
    ([128, ...] partition-major, contiguous per partition) so every DMA is
    large-descriptor and the single queue is bandwidth- not
    descriptor-rate-bound.  Weights are split per h-chunk so compute starts
    after ~1/3 of the bytes.
  * Biases bq/bv are folded into QPs/VP at projection time; bl cancels in
    softmax.  Logits are bounded (|logit| <= sum|Wl|), so softmax skips the
    max-subtraction pass.

Per-core dataflow (h on partitions for the hot loop), phased per h-chunk:
  QPs[hc] [128(h), 512(b,ns)] = Wq-slice.T @ qT + bq   (PE psum, DVE copy)
  VP[hc]  [128(h), S(slots)]  = Wv-slice.T @ vG + bv   (S = K0+K1)
  JT      [128, strip(kk)*256] bf16 = QPs-half + VP[slot]  (DVE)
  tanh in-place on JT (one ACT op per slot-group)
  logits  psum: batch b uses cols b*256:(b+1)*256 and PE col-strips
          (0, 32) for b0 / (64, 96) for b1 (disjoint psum partition rows:
          start=True zeroes the whole 2KB bank row), accumulated over hc
          with zero-padded Wl lhsT (pair j, j+P shares lhsT; tile_position).
  softmax over slots after PE-transposing logits to [ns, slots];
  hc3 runs b1 before b0 so b1's softmax hides under b0's tanh stream.
"""

import os
import sys

import numpy as np

sys.path.insert(0, "/opt/trn_rl_repo")

import concourse.bass as bass
import concourse.mybir as mybir
from concourse import bacc, bass_utils
from concourse.tile import TileContext

# Problem shapes (hardcoded per contract -- kernel.py must be self-contained)
B, N, S, K = 16, 4, 64, 50
VD, QD, H = 1024, 768, 512
NCORES = 8
BPC = B // NCORES          # batches per core = 2
NS = BPC * N * S           # 512 rows (b, n, s) per core
NSB = NS // BPC            # 256 rows per batch
HC = H // 128              # 4 h-chunks
QC = QD // 128             # 6 qd-chunks
VC = VD // 128             # 8 vd-chunks

F32 = mybir.dt.float32
BF16 = mybir.dt.bfloat16

_CACHE = {}


def _groups(hc, b, P):
    """Pair-index groups for (hc, batch).  First groups of the whole kernel
    are small so the first tanh issues as early as possible; afterwards one
    big group per (hc, b) amortizes the ACT per-op bubble."""
    pairs = list(range(P))
    if hc == 0 and b == 0:
        gs = [pairs[0:2], pairs[2:9], pairs[9:]]
    elif hc == 0:
        h = (P + 1) // 2
        gs = [pairs[:h], pairs[h:]]
    else:
        gs = [pairs]
    return [g for g in gs if g]


def _build_nc(K0, K1):
    P0, P1 = K0 // 2, K1 // 2
    SL = K0 + K1               # slots per core
    KMAX = max(K0, K1)
    WZ0 = P0 * P0              # wlz cols per hc for batch 0
    WZC = WZ0 + P1 * P1        # wlz cols per hc total

    nc = bacc.Bacc("TRN2", target_bir_lowering=False)

    # All inputs are pre-packed on host into SBUF layout [128, ...]
    qT_h = nc.dram_tensor("qT", [128, QC * NS], BF16, kind="ExternalInput")
    vG_h = nc.dram_tensor("vG", [128, VC * SL], BF16, kind="ExternalInput")
    wq_h = [
        nc.dram_tensor(f"wq{hc}", [128, QC * 128], BF16, kind="ExternalInput")
        for hc in range(HC)
    ]
    wv_h = [
        nc.dram_tensor(f"wv{hc}", [128, VC * 128], BF16, kind="ExternalInput")
        for hc in range(HC)
    ]
    # packed [128, 12]: cols 0:4 Wl chunks, 4:8 bq chunks, 8:12 bv chunks
    wlb_h = nc.dram_tensor("wlb", [128, 12], F32, kind="ExternalInput")
    # zero-padded Wl variants, per (hc, b, j): [128, Pb] slab, col c = Wl*(c==j)
    wlz_h = nc.dram_tensor("wlz", [128, HC * WZC], BF16, kind="ExternalInput")
    # additive mask: col s = 0.0 for a real slot, -1e9 for padding/masked
    msk_h = nc.dram_tensor("msk", [128, SL], F32, kind="ExternalInput")
    id_h = nc.dram_tensor("ident", [128, 128], F32, kind="ExternalInput")
    # out col (nsc, j): w[ns = nsc*128 + p, slot j]
    out_h = nc.dram_tensor(
        "out", [128, (NS // 128) * KMAX], F32, kind="ExternalOutput"
    )

    with TileContext(nc) as tc:
        with (
            tc.tile_pool(name="persist", bufs=1) as pp,
            tc.tile_pool(name="ppsum", bufs=1, space="PSUM") as ppsum,
            tc.tile_pool(name="projps", bufs=2, space="PSUM") as pjps,
            tc.tile_pool(name="smpsum", bufs=2, space="PSUM") as sps,
        ):
            # ---- DMA loads.  qT is packed batch-half-major ([128, bh, c, j])
            # so the b0 projection chain starts after ~1MB of DMA ----
            wlb = pp.tile_from(wlb_h[:, :], name="wlb")
            qts = pp.tile([128, 2, QC, NSB], BF16, name="qts")
            qts_f = qts[:, :, :, :].rearrange("p h c j -> p (h c j)")
            HB = QC * NSB
            wqt = [None] * HC
            wvt = [None] * HC
            vts = pp.tile_from(vG_h[:, :], name="vts")
            wvt[0] = pp.tile_from(wv_h[0][:, :], name="wv0")
            nc.sync.dma_start(qts_f[:, 0:HB], qT_h[:, 0:HB])
            wqt[0] = pp.tile_from(wq_h[0][:, :], name="wq0")
            nc.sync.dma_start(qts_f[:, HB : 2 * HB], qT_h[:, HB : 2 * HB])
            wqt[1] = pp.tile_from(wq_h[1][:, :], name="wq1")
            wvt[1] = pp.tile_from(wv_h[1][:, :], name="wv1")
            wlz = pp.tile_from(wlz_h[:, :], name="wlz")
            msk = pp.tile_from(msk_h[:, :], name="msk")
            ident = pp.tile_from(id_h[:, :], name="ident")
            wqt[2] = pp.tile_from(wq_h[2][:, :], name="wq2")
            wvt[2] = pp.tile_from(wv_h[2][:, :], name="wv2")
            wqt[3] = pp.tile_from(wq_h[3][:, :], name="wq3")
            wvt[3] = pp.tile_from(wv_h[3][:, :], name="wv3")

            # qp (all h-chunks): [128, (hc, b, ns)] bf16, +bq folded
            QPs = pp.tile([128, HC * NS], BF16, name="QPs")
            # vp slot table: [128, (hc, slot)] f32, +bv folded
            VP = pp.tile([128, HC * SL], F32, name="VP")

            # logits psum: batch b owns cols b*256:(b+1)*256 and PE col-strips
            # (0, 32) for b0 / (64, 96) for b1 -> psum rows 0:P0, 32:32+P0,
            # 64:64+P1, 96:96+P1.  Strips of the two batches must not share
            # psum partition rows: start=True zeroes the whole 2KB bank row.
            ps_log = ppsum.tile([128, NS], F32, name="ps_log")

            def proj_phase(hc):
                """Compute QPs/VP h-chunk hc.  Projections, then the bias-fold
                copies, run per batch-half so b0's QPs is ready before b1's
                qT half has even arrived (hc0 startup)."""
                pv = pjps.tile([128, SL], F32, tag="pv", name="pv")
                for vc in range(VC):
                    nc.tensor.matmul(
                        pv[:, :],
                        wvt[hc][:, vc * 128 : (vc + 1) * 128],
                        vts[:, vc * SL : (vc + 1) * SL],
                        start=(vc == 0),
                        stop=(vc == VC - 1),
                    )
                nc.vector.tensor_scalar_add(
                    VP[:, hc * SL : (hc + 1) * SL],
                    pv[:, :],
                    wlb[:, 2 * HC + hc : 2 * HC + hc + 1],
                )
                pq = pjps.tile([128, NS], F32, tag="pq", name="pq")
                for bh in range(2):
                    for qc in range(QC):
                        nc.tensor.matmul(
                            pq[:, bh * NSB : (bh + 1) * NSB],
                            wqt[hc][:, qc * 128 : (qc + 1) * 128],
                            qts[:, bh, qc, :],
                            start=(qc == 0),
                            stop=(qc == QC - 1),
                        )
                    nc.vector.tensor_scalar_add(
                        QPs[:, hc * NS + bh * NSB : hc * NS + (bh + 1) * NSB],
                        pq[:, bh * NSB : (bh + 1) * NSB],
                        wlb[:, HC + hc : HC + hc + 1],
                    )

            def main_hc(hc, b, mp, mid_cb=None):
                """Joint tanh + logit matmuls for one (h-chunk, batch)."""
                P = P0 if b == 0 else P1
                wzb = hc * WZC + (0 if b == 0 else WZ0)
                vcb = hc * SL + b * K0
                qpo = hc * NS + b * NSB
                groups = _groups(hc, b, P)
                mid_g = min(1, len(groups) - 1)
                for g, js in enumerate(groups):
                    if g == mid_g and mid_cb is not None:
                        mid_cb()
                    L = len(js)
                    JT = mp.tile([128, 2 * L * NSB], BF16, tag="JT", name="JT")
                    for kk in range(2 * L):
                        slot = js[kk] if kk < L else js[kk - L] + P
                        nc.vector.tensor_scalar_add(
                            JT[:, kk * NSB : (kk + 1) * NSB],
                            QPs[:, qpo : qpo + NSB],
                            VP[:, vcb + slot : vcb + slot + 1],
                        )
                    # in-place tanh over the whole group
                    nc.scalar.activation(
                        JT[:, :], JT[:, :], mybir.ActivationFunctionType.Tanh
                    )
                    bcs = slice(b * NSB, (b + 1) * NSB)
                    r0 = 64 * b
                    r1 = r0 + 32
                    for jj, j in enumerate(js):
                        first = hc == 0 and g == 0 and jj == 0
                        last = hc == HC - 1 and g == len(groups) - 1 and jj == L - 1
                        nc.tensor.matmul(
                            ps_log[r0 : r0 + P, bcs],
                            wlz[:, wzb + j * P : wzb + (j + 1) * P],
                            JT[:, jj * NSB : (jj + 1) * NSB],
                            start=first,
                            stop=last,
                            tile_position=(0, r0),
                            skip_group_check=True,
                        )
                        nc.tensor.matmul(
                            ps_log[r1 : r1 + P, bcs],
                            wlz[:, wzb + j * P : wzb + (j + 1) * P],
                            JT[:, (L + jj) * NSB : (L + jj + 1) * NSB],
                            start=first,
                            stop=last,
                            tile_position=(0, r1),
                            skip_group_check=True,
                        )

            LGA = pp.tile([96 + 32, NSB], F32, name="LGA")
            W_all = pp.tile([128, NS // 128, KMAX], F32, name="W_all")

            def softmax_b(b):
                """Masked softmax for batch b (no max-pass: |logits| <~ 1.5)."""
                P = P0 if b == 0 else P1
                Kb = 2 * P
                r0 = 64 * b
                r1 = r0 + 32
                bcs = slice(b * NSB, (b + 1) * NSB)
                nc.vector.tensor_copy(LGA[r0 : r0 + P, :], ps_log[r0 : r0 + P, bcs])
                nc.vector.tensor_copy(LGA[r1 : r1 + P, :], ps_log[r1 : r1 + P, bcs])
                for nsb in range(NSB // 128):
                    nsc = b * 2 + nsb
                    ps_t = sps.tile([128, KMAX], F32, tag="ps_t", name="ps_t")
                    nc.tensor.transpose(
                        ps_t[:, 0:P],
                        LGA[r0 : r0 + P, nsb * 128 : (nsb + 1) * 128],
                        ident[r0 : r0 + P, r0 : r0 + P],
                        tile_position=(r0, 0),
                    )
                    nc.tensor.transpose(
                        ps_t[:, P : 2 * P],
                        LGA[r1 : r1 + P, nsb * 128 : (nsb + 1) * 128],
                        ident[r1 : r1 + P, r1 : r1 + P],
                        tile_position=(r1, 0),
                    )
                    LT = pp.tile([128, KMAX], F32, name=f"LT{nsc}")
                    nc.vector.tensor_add(
                        LT[:, 0:Kb], ps_t[:, 0:Kb], msk[:, b * K0 : b * K0 + Kb]
                    )
                    EX = pp.tile([128, KMAX], F32, name=f"EX{nsc}")
                    sm = pp.tile([128, 1], F32, name=f"sm{nsc}")
                    nc.scalar.activation(
                        EX[:, 0:Kb], LT[:, 0:Kb],
                        mybir.ActivationFunctionType.Exp,
                        accum_out=sm[:, 0:1],
                    )
                    rs = pp.tile([128, 1], F32, name=f"rs{nsc}")
                    nc.vector.reciprocal(rs[:, :], sm[:, :])
                    nc.vector.tensor_scalar_mul(
                        W_all[:, nsc, 0:Kb], EX[:, 0:Kb], rs[:, 0:1]
                    )
                    if Kb < KMAX:
                        nc.vector.memset(W_all[:, nsc, Kb:KMAX], 0.0)
                out_v = out_h[:, :].rearrange("p (c j) -> p c j", j=KMAX)
                nc.sync.dma_start(
                    out_v[:, 2 * b : 2 * b + 2, :], W_all[:, 2 * b : 2 * b + 2, :]
                )

            proj_phase(0)
            with tc.tile_pool(name="main", bufs=4) as mp:

                def prio(hc):
                    def cb():
                        with tc.high_priority():
                            proj_phase(hc)

                    return cb

                main_hc(0, 0, mp, mid_cb=prio(1))
                main_hc(0, 1, mp, mid_cb=prio(2))
                main_hc(1, 0, mp, mid_cb=prio(3))
                main_hc(1, 1, mp)
                main_hc(2, 0, mp)
                main_hc(2, 1, mp)
                main_hc(3, 1, mp)   # b1 first: its softmax hides under b0 tanh
                softmax_b(1)
                main_hc(3, 0, mp)
                softmax_b(0)

    nc.finalize()
    return nc


def _plan(box_mask):
    """Pair batches onto cores large-with-small; return per-core batch ids,
    per-batch unmasked box index lists, and padded slot counts (K0, K1)."""
    mask = np.asarray(box_mask) > 0
    counts = mask.sum(axis=1)
    order = np.argsort(-counts, kind="stable")
    batA = order[:NCORES]
    batB = order[NCORES:][::-1]
    kidx = [np.nonzero(mask[b])[0] for b in range(B)]

    def pad_even(n):
        n = max(int(n), 2)
        return n + (n & 1)

    K0 = pad_even(counts[batA].max())
    K1 = pad_even(counts[batB].max())
    return batA, batB, kidx, K0, K1


def _dev_pack(M):
    """[C*128, X] row-major -> [128, C*X] partition-major (SBUF layout)."""
    D, X = M.shape
    C = D // 128
    return np.ascontiguousarray(
        M.reshape(C, 128, X).transpose(1, 0, 2).reshape(128, C * X)
    )


def _prep_in_maps(v, q, box_mask, Wv, bv, Wq, bq, Wl, plan):
    """Host-side layout prep: gather unmasked boxes, shard over cores,
    pack into device layouts."""
    import ml_dtypes

    batA, batB, kidx, K0, K1 = plan
    P0, P1 = K0 // 2, K1 // 2
    SL = K0 + K1
    WZ0 = P0 * P0
    WZC = WZ0 + P1 * P1

    v = np.asarray(v, np.float32).reshape(B, K, VD)
    q = np.asarray(q, np.float32).reshape(B, N * S, QD)

    WqT = np.asarray(Wq, np.float32).T                                # [QD, H]
    WvT = np.asarray(Wv, np.float32).T                                # [VD, H]
    shared = {}
    for hc in range(HC):
        shared[f"wq{hc}"] = _dev_pack(
            np.ascontiguousarray(WqT[:, hc * 128 : (hc + 1) * 128])
        ).astype(ml_dtypes.bfloat16)
        shared[f"wv{hc}"] = _dev_pack(
            np.ascontiguousarray(WvT[:, hc * 128 : (hc + 1) * 128])
        ).astype(ml_dtypes.bfloat16)
    wlb = np.zeros((128, 12), np.float32)
    wl_chunks = np.asarray(Wl, np.float32).reshape(HC, 128).T         # [128, hc]
    wlb[:, 0:4] = wl_chunks
    wlb[:, 4:8] = np.asarray(bq, np.float32).reshape(HC, 128).T
    wlb[:, 8:12] = np.asarray(bv, np.float32).reshape(HC, 128).T
    shared["wlb"] = wlb
    # zero-padded Wl variants per (hc, b, j)
    wlz = np.zeros((128, HC, WZC), np.float32)
    for j in range(P0):
        wlz[:, :, j * P0 + j] = wl_chunks
    for j in range(P1):
        wlz[:, :, WZ0 + j * P1 + j] = wl_chunks
    shared["wlz"] = wlz.reshape(128, HC * WZC).astype(ml_dtypes.bfloat16)
    shared["ident"] = np.eye(128, dtype=np.float32)

    in_maps = []
    for c in range(NCORES):
        qc = np.stack([q[batA[c]], q[batB[c]]])                       # [2,NSB,QD]
        # [128, (bh, c, j)]: batch-half-major so b0's projections need only
        # the first half of the transfer
        qT = (
            qc.transpose(2, 0, 1)                                     # [QD,2,NSB]
            .reshape(QC, 128, 2, NSB)
            .transpose(1, 2, 0, 3)
            .reshape(128, 2 * QC * NSB)
        )
        vg = np.zeros((SL, VD), np.float32)
        moff = np.full((1, SL), -1e9, np.float32)
        for bi, Kb, off in [(batA[c], K0, 0), (batB[c], K1, K0)]:
            ks = kidx[bi]
            vg[off : off + len(ks)] = v[bi, ks]
            moff[0, off : off + len(ks)] = 0.0
        vG = np.ascontiguousarray(vg.T)                               # [VD, SL]
        msk = np.ascontiguousarray(np.broadcast_to(moff, (128, SL)))
        in_maps.append(
            {
                "qT": np.ascontiguousarray(qT).astype(ml_dtypes.bfloat16),
                "vG": _dev_pack(vG).astype(ml_dtypes.bfloat16),
                "msk": msk,
                **shared,
            }
        )
    return in_maps


def kernel(v, q, box_mask, tags_attention, Wv, bv, Wq, bq, Wl, bl):
    # bl shifts all unmasked logits uniformly -> cancels in softmax.
    # tags_attention is unused by the reference module.
    plan = _plan(box_mask)
    batA, batB, kidx, K0, K1 = plan
    KMAX = max(K0, K1)
    key = (K0, K1)
    if key not in _CACHE:
        _CACHE[key] = _build_nc(K0, K1)
    nc = _CACHE[key]
    in_maps = _prep_in_maps(v, q, box_mask, Wv, bv, Wq, bq, Wl, plan)
    res = bass_utils.run_bass_kernel_spmd(
        nc,
        in_maps,
        core_ids=list(range(NCORES)),
        trace=bool(os.environ.get("KERNEL_TRACE")),
        tmpdir=os.environ.get("KERNEL_TMPDIR"),
    )
    _CACHE["last_result"] = res
    out = np.zeros((B, N * S, K), np.float32)
    for c in range(NCORES):
        w = (
            res.results[c]["out"]
            .reshape(128, NS // 128, KMAX)
            .transpose(1, 0, 2)
            .reshape(NS, KMAX)
        )
        for bi, off in [(batA[c], 0), (batB[c], NSB)]:
            ks = kidx[bi]
            if len(ks) == 0:
                out[bi, :, :] = 1.0 / K
            else:
                out[bi, :, ks] = w[off : off + NSB, : len(ks)].T
    return out.reshape(B, N, S, K)


# revision 28
# speedup vs baseline: 4.5005x; 1.0470x over previous
"""Fused additive-attention kernel for Trainium2 (8 NeuronCores, SPMD).

Computes  w = softmax_K( mask ? (Wl . tanh(vW_v^T + qW_q^T) + bl) : -1e9 )
without materializing the [B,N,S,K,H] joint_repr intermediate.

Key ideas over the naive formulation:
  * Masked boxes get weight exactly 0 (exp(-1e9) underflows), so only the
    unmasked boxes are computed.  Host gathers each batch's unmasked box
    list; batches are paired onto cores large-with-small so the padded
    per-slot counts (K0 for the core's first batch, K1 for its second) stay
    near the true max.  Masked/padding slots are -1e9'd on device and the
    host scatters results back into the full [B,N,S,K] output (zeros for
    masked boxes).
  * The broadcast add vp[b,k,h] + qp[b,n,s,h] runs as DVE tensor_scalar_add
    with a per-partition [128,1] vp operand (high DVE perf mode).
  * All device tensors are packed on host into their SBUF layout
    ([128, ...] partition-major, contiguous per partition) so every DMA is
    large-descriptor and the single queue is bandwidth- not
    descriptor-rate-bound.  Weights are split per h-chunk so compute starts
    after ~1/3 of the bytes.
  * Biases bq/bv are folded into QPs/VP at projection time; bl cancels in
    softmax.  Logits are bounded (|logit| <= sum|Wl|), so softmax skips the
    max-subtraction pass.

Per-core dataflow (h on partitions for the hot loop), phased per h-chunk:
  QPs[hc] [128(h), 512(b,ns)] = Wq-slice.T @ qT + bq   (PE psum, DVE copy)
  VP[hc]  [128(h), S(slots)]  = Wv-slice.T @ vG + bv   (S = K0+K1)
  JT      [128, strip(kk)*256] bf16 = QPs-half + VP[slot]  (DVE)
  tanh in-place on JT (one ACT op per slot-group)
  logits  psum: batch b uses cols b*256:(b+1)*256 and PE col-strips
          (0, 32) for b0 / (64, 96) for b1 (disjoint psum partition rows:
          start=True zeroes the whole 2KB bank row), accumulated over hc
          with zero-padded Wl lhsT (pair j, j+P shares lhsT; tile_position).
  softmax over slots after PE-transposing logits to [ns, slots];
  hc3 runs b1 before b0 so b1's softmax hides under b0's tanh stream.
"""

import os
import sys

import numpy as np

sys.path.insert(0, "/opt/trn_rl_repo")

import concourse.bass as bass
import concourse.mybir as mybir
from concourse import bacc, bass_utils
from concourse.tile import TileContext

# Problem shapes (hardcoded per contract -- kernel.py must be self-contained)
B, N, S, K = 16, 4, 64, 50
VD, QD, H = 1024, 768, 512
NCORES = 8
BPC = B // NCORES          # batches per core = 2
NS = BPC * N * S           # 512 rows (b, n, s) per core
NSB = NS // BPC            # 256 rows per batch
HC = H // 128              # 4 h-chunks
QC = QD // 128             # 6 qd-chunks
VC = VD // 128             # 8 vd-chunks

F32 = mybir.dt.float32
BF16 = mybir.dt.bfloat16

_CACHE = {}


def _groups(hc, b, P):
    """Pair-index groups for (hc, batch).  First groups of the whole kernel
    are small so the first tanh issues as early as possible; the very last
    (hc3, b0) group is small so the tail logit-matmul burst after the final
    tanh is short."""
    pairs = list(range(P))
    if hc == 0 and b == 0:
        gs = [pairs[0:2], pairs[2:9], pairs[9:]]
    elif hc == HC - 1 and b == 0:
        gs = [pairs[: P // 2], pairs[P // 2 : -3], pairs[-3:]]
    else:
        h = (P + 1) // 2
        gs = [pairs[:h], pairs[h:]]
    return [g for g in gs if g]


def _build_nc(K0, K1):
    P0, P1 = K0 // 2, K1 // 2
    SL = K0 + K1               # slots per core
    KMAX = max(K0, K1)
    WZ0 = P0 * P0              # wlz cols per hc for batch 0
    WZC = WZ0 + P1 * P1        # wlz cols per hc total

    nc = bacc.Bacc("TRN2", target_bir_lowering=False)

    # All inputs are pre-packed on host into SBUF layout [128, ...] and
    # merged into a few "blob" tensors so each is one large-descriptor DMA.
    QW = QC * 128              # wq slab cols
    VW = VC * 128              # wv slab cols
    QH = QC * NSB              # qT batch-half cols
    VGW = VC * SL
    # blobA: [vG | wv0]   blobB: [qT-half0 | wq0]
    # blobC: [qT-half1 | wq1 | wv1]   blobD: [wlz | wq2 | wv2 | wq3 | wv3]
    blA_h = nc.dram_tensor("blobA", [128, VGW + VW], BF16, kind="ExternalInput")
    blB_h = nc.dram_tensor("blobB", [128, QH + QW], BF16, kind="ExternalInput")
    blC_h = nc.dram_tensor(
        "blobC", [128, QH + QW + VW], BF16, kind="ExternalInput"
    )
    blD_h = nc.dram_tensor(
        "blobD", [128, HC * WZC + 2 * (QW + VW)], BF16, kind="ExternalInput"
    )
    # packed [128, 12]: cols 0:4 Wl chunks, 4:8 bq chunks, 8:12 bv+bq chunks
    wlb_h = nc.dram_tensor("wlb", [128, 12], F32, kind="ExternalInput")
    # additive mask: col s = 0.0 for a real slot, -1e9 for padding/masked
    msk_h = nc.dram_tensor("msk", [128, SL], F32, kind="ExternalInput")
    id_h = nc.dram_tensor("ident", [128, 128], F32, kind="ExternalInput")
    # out col (nsc, j): w[ns = nsc*128 + p, slot j]
    out_h = nc.dram_tensor(
        "out", [128, (NS // 128) * KMAX], F32, kind="ExternalOutput"
    )

    with TileContext(nc) as tc:
        with (
            tc.tile_pool(name="persist", bufs=1) as pp,
            tc.tile_pool(name="ppsum", bufs=1, space="PSUM") as ppsum,
            tc.tile_pool(name="projps", bufs=2, space="PSUM") as pjps,
            tc.tile_pool(name="smpsum", bufs=2, space="PSUM") as sps,
        ):
            # ---- DMA loads.  blobB (b0's qT half + wq0) streams on the
            # Activation engine's DGE queue in parallel with blobA on the
            # sync queue, so the b0 projection chain starts after ~600KB ----
            blB = pp.tile([128, QH + QW], BF16, name="blB")
            nc.scalar.dma_start(blB[:, :], blB_h[:, :])
            wlb = pp.tile_from(wlb_h[:, :], name="wlb")
            blA = pp.tile_from(blA_h[:, :], name="blA")
            blC = pp.tile([128, QH + QW + VW], BF16, name="blC")
            nc.scalar.dma_start(blC[:, :], blC_h[:, :])
            blD = pp.tile_from(blD_h[:, :], name="blD")
            msk = pp.tile_from(msk_h[:, :], name="msk")
            ident = pp.tile_from(id_h[:, :], name="ident")

            vts = blA[:, 0:VGW]
            qtv = [blB[:, 0:QH], blC[:, 0:QH]]           # per batch-half
            wqt = [
                blB[:, QH : QH + QW],
                blC[:, QH : QH + QW],
                blD[:, HC * WZC : HC * WZC + QW],
                blD[:, HC * WZC + QW + VW : HC * WZC + 2 * QW + VW],
            ]
            wvt = [
                blA[:, VGW : VGW + VW],
                blC[:, QH + QW : QH + QW + VW],
                blD[:, HC * WZC + QW : HC * WZC + QW + VW],
                blD[:, HC * WZC + 2 * QW + VW : HC * WZC + 2 * (QW + VW)],
            ]
            wlz = blD[:, 0 : HC * WZC]

            # qp (all h-chunks): [128, (hc, b, ns)] bf16, +bq folded
            QPs = pp.tile([128, HC * NS], BF16, name="QPs")
            # vp slot table: [128, (hc, slot)] f32, +bv folded
            VP = pp.tile([128, HC * SL], F32, name="VP")

            # logits psum: batch b owns cols b*256:(b+1)*256 and PE col-strips
            # (0, 32) for b0 / (64, 96) for b1 -> psum rows 0:P0, 32:32+P0,
            # 64:64+P1, 96:96+P1.  Strips of the two batches must not share
            # psum partition rows: start=True zeroes the whole 2KB bank row.
            ps_log = ppsum.tile([128, NS], F32, name="ps_log")

            def proj_phase(hc):
                """Compute QPs/VP h-chunk hc.  Projections and copies run per
                batch-half so b0's QPs is ready before b1's qT half has even
                arrived (hc0 startup).  bq is folded into VP (wlb cols 8:12
                hold bv+bq), so the QPs copy is a pure cast and all but the
                latency-critical first one run as gpsimd casting DMAs,
                keeping DVE free for the broadcast adds."""
                pv = pjps.tile([128, SL], F32, tag="pv", name="pv")
                for vc in range(VC):
                    nc.tensor.matmul(
                        pv[:, :],
                        wvt[hc][:, vc * 128 : (vc + 1) * 128],
                        vts[:, vc * SL : (vc + 1) * SL],
                        start=(vc == 0),
                        stop=(vc == VC - 1),
                    )
                nc.vector.tensor_scalar_add(
                    VP[:, hc * SL : (hc + 1) * SL],
                    pv[:, :],
                    wlb[:, 2 * HC + hc : 2 * HC + hc + 1],
                )
                pq = pjps.tile([128, NS], F32, tag="pq", name="pq")
                for bh in range(2):
                    for qc in range(QC):
                        nc.tensor.matmul(
                            pq[:, bh * NSB : (bh + 1) * NSB],
                            wqt[hc][:, qc * 128 : (qc + 1) * 128],
                            qtv[bh][:, qc * NSB : (qc + 1) * NSB],
                            start=(qc == 0),
                            stop=(qc == QC - 1),
                        )
                    nc.vector.tensor_copy(
                        QPs[:, hc * NS + bh * NSB : hc * NS + (bh + 1) * NSB],
                        pq[:, bh * NSB : (bh + 1) * NSB],
                    )

            def main_hc(hc, b, mp, mid_cb=None):
                """Joint tanh + logit matmuls for one (h-chunk, batch)."""
                P = P0 if b == 0 else P1
                wzb = hc * WZC + (0 if b == 0 else WZ0)
                vcb = hc * SL + b * K0
                qpo = hc * NS + b * NSB
                groups = _groups(hc, b, P)
                mid_g = min(1, len(groups) - 1)
                for g, js in enumerate(groups):
                    if g == mid_g and mid_cb is not None:
                        mid_cb()
                    L = len(js)
                    JT = mp.tile([128, 2 * L * NSB], BF16, tag="JT", name="JT")
                    for kk in range(2 * L):
                        slot = js[kk] if kk < L else js[kk - L] + P
                        nc.vector.tensor_scalar_add(
                            JT[:, kk * NSB : (kk + 1) * NSB],
                            QPs[:, qpo : qpo + NSB],
                            VP[:, vcb + slot : vcb + slot + 1],
                        )
                    # in-place tanh over the whole group
                    nc.scalar.activation(
                        JT[:, :], JT[:, :], mybir.ActivationFunctionType.Tanh
                    )
                    bcs = slice(b * NSB, (b + 1) * NSB)
                    r0 = 64 * b
                    r1 = r0 + 32
                    for jj, j in enumerate(js):
                        first = hc == 0 and g == 0 and jj == 0
                        last = hc == HC - 1 and g == len(groups) - 1 and jj == L - 1
                        nc.tensor.matmul(
                            ps_log[r0 : r0 + P, bcs],
                            wlz[:, wzb + j * P : wzb + (j + 1) * P],
                            JT[:, jj * NSB : (jj + 1) * NSB],
                            start=first,
                            stop=last,
                            tile_position=(0, r0),
                            skip_group_check=True,
                        )
                        nc.tensor.matmul(
                            ps_log[r1 : r1 + P, bcs],
                            wlz[:, wzb + j * P : wzb + (j + 1) * P],
                            JT[:, (L + jj) * NSB : (L + jj + 1) * NSB],
                            start=first,
                            stop=last,
                            tile_position=(0, r1),
                            skip_group_check=True,
                        )

            LGA = pp.tile([96 + 32, NSB], F32, name="LGA")
            W_all = pp.tile([128, NS // 128, KMAX], F32, name="W_all")

            def softmax_b(b):
                """Masked softmax for batch b (no max-pass: |logits| <~ 1.5)."""
                P = P0 if b == 0 else P1
                Kb = 2 * P
                r0 = 64 * b
                r1 = r0 + 32
                bcs = slice(b * NSB, (b + 1) * NSB)
                nc.vector.tensor_copy(LGA[r0 : r0 + P, :], ps_log[r0 : r0 + P, bcs])
                nc.vector.tensor_copy(LGA[r1 : r1 + P, :], ps_log[r1 : r1 + P, bcs])
                for nsb in range(NSB // 128):
                    nsc = b * 2 + nsb
                    ps_t = sps.tile([128, KMAX], F32, tag="ps_t", name="ps_t")
                    nc.tensor.transpose(
                        ps_t[:, 0:P],
                        LGA[r0 : r0 + P, nsb * 128 : (nsb + 1) * 128],
                        ident[r0 : r0 + P, r0 : r0 + P],
                        tile_position=(r0, 0),
                    )
                    nc.tensor.transpose(
                        ps_t[:, P : 2 * P],
                        LGA[r1 : r1 + P, nsb * 128 : (nsb + 1) * 128],
                        ident[r1 : r1 + P, r1 : r1 + P],
                        tile_position=(r1, 0),
                    )
                    LT = pp.tile([128, KMAX], F32, name=f"LT{nsc}")
                    nc.vector.tensor_add(
                        LT[:, 0:Kb], ps_t[:, 0:Kb], msk[:, b * K0 : b * K0 + Kb]
                    )
                    EX = pp.tile([128, KMAX], F32, name=f"EX{nsc}")
                    sm = pp.tile([128, 1], F32, name=f"sm{nsc}")
                    nc.scalar.activation(
                        EX[:, 0:Kb], LT[:, 0:Kb],
                        mybir.ActivationFunctionType.Exp,
                        accum_out=sm[:, 0:1],
                    )
                    rs = pp.tile([128, 1], F32, name=f"rs{nsc}")
                    nc.vector.reciprocal(rs[:, :], sm[:, :])
                    nc.vector.tensor_scalar_mul(
                        W_all[:, nsc, 0:Kb], EX[:, 0:Kb], rs[:, 0:1]
                    )
                    if Kb < KMAX:
                        nc.vector.memset(W_all[:, nsc, Kb:KMAX], 0.0)
                    out_v = out_h[:, :].rearrange("p (c j) -> p c j", j=KMAX)
                    nc.sync.dma_start(
                        out_v[:, nsc : nsc + 1, :], W_all[:, nsc : nsc + 1, :]
                    )

            proj_phase(0)
            with tc.tile_pool(name="main", bufs=4) as mp:

                def prio(hc):
                    def cb():
                        with tc.high_priority():
                            proj_phase(hc)

                    return cb

                main_hc(0, 0, mp, mid_cb=prio(1))
                main_hc(0, 1, mp, mid_cb=prio(2))
                main_hc(1, 0, mp, mid_cb=prio(3))
                main_hc(1, 1, mp)
                main_hc(2, 0, mp)
                main_hc(2, 1, mp)
                main_hc(3, 1, mp)   # b1 first: its softmax hides under b0 tanh
                softmax_b(1)
                main_hc(3, 0, mp)
                softmax_b(0)

    nc.finalize()
    return nc


def _plan(box_mask):
    """Pair batches onto cores large-with-small; return per-core batch ids,
    per-batch unmasked box index lists, and padded slot counts (K0, K1)."""
    mask = np.asarray(box_mask) > 0
    counts = mask.sum(axis=1)
    order = np.argsort(-counts, kind="stable")
    batA = order[:NCORES]
    batB = order[NCORES:][::-1]
    kidx = [np.nonzero(mask[b])[0] for b in range(B)]

    def pad_even(n):
        n = max(int(n), 2)
        return n + (n & 1)

    K0 = pad_even(counts[batA].max())
    K1 = pad_even(counts[batB].max())
    return batA, batB, kidx, K0, K1


def _dev_pack(M):
    """[C*128, X] row-major -> [128, C*X] partition-major (SBUF layout)."""
    D, X = M.shape
    C = D // 128
    return np.ascontiguousarray(
        M.reshape(C, 128, X).transpose(1, 0, 2).reshape(128, C * X)
    )


def _prep_in_maps(v, q, box_mask, Wv, bv, Wq, bq, Wl, plan):
    """Host-side layout prep: gather unmasked boxes, shard over cores,
    pack into device layouts."""
    import ml_dtypes

    batA, batB, kidx, K0, K1 = plan
    P0, P1 = K0 // 2, K1 // 2
    SL = K0 + K1
    WZ0 = P0 * P0
    WZC = WZ0 + P1 * P1

    v = np.asarray(v, np.float32).reshape(B, K, VD)
    q = np.asarray(q, np.float32).reshape(B, N * S, QD)

    WqT = np.asarray(Wq, np.float32).T                                # [QD, H]
    WvT = np.asarray(Wv, np.float32).T                                # [VD, H]
    bf16 = ml_dtypes.bfloat16
    wq = [
        _dev_pack(np.ascontiguousarray(WqT[:, hc * 128 : (hc + 1) * 128]))
        for hc in range(HC)
    ]
    wv = [
        _dev_pack(np.ascontiguousarray(WvT[:, hc * 128 : (hc + 1) * 128]))
        for hc in range(HC)
    ]
    wlb = np.zeros((128, 12), np.float32)
    wl_chunks = np.asarray(Wl, np.float32).reshape(HC, 128).T         # [128, hc]
    wlb[:, 0:4] = wl_chunks
    wlb[:, 4:8] = np.asarray(bq, np.float32).reshape(HC, 128).T
    # VP carries both biases (bv+bq) so the QPs copy is a pure cast
    wlb[:, 8:12] = (
        np.asarray(bv, np.float32) + np.asarray(bq, np.float32)
    ).reshape(HC, 128).T
    shared = {"wlb": wlb}
    # zero-padded Wl variants per (hc, b, j)
    wlz = np.zeros((128, HC, WZC), np.float32)
    for j in range(P0):
        wlz[:, :, j * P0 + j] = wl_chunks
    for j in range(P1):
        wlz[:, :, WZ0 + j * P1 + j] = wl_chunks
    wlz = wlz.reshape(128, HC * WZC)
    shared["blobD"] = np.concatenate(
        [wlz, wq[2], wv[2], wq[3], wv[3]], axis=1
    ).astype(bf16)
    shared["ident"] = np.eye(128, dtype=np.float32)

    in_maps = []
    for c in range(NCORES):
        qc = np.stack([q[batA[c]], q[batB[c]]])                       # [2,NSB,QD]
        # [128, (bh, c, j)]: batch-half-major so b0's projections need only
        # the first half of the transfer
        qT = (
            qc.transpose(2, 0, 1)                                     # [QD,2,NSB]
            .reshape(QC, 128, 2, NSB)
            .transpose(1, 2, 0, 3)
            .reshape(128, 2 * QC * NSB)
        )
        vg = np.zeros((SL, VD), np.float32)
        moff = np.full((1, SL), -1e9, np.float32)
        for bi, Kb, off in [(batA[c], K0, 0), (batB[c], K1, K0)]:
            ks = kidx[bi]
            vg[off : off + len(ks)] = v[bi, ks]
            moff[0, off : off + len(ks)] = 0.0
        vG = np.ascontiguousarray(vg.T)                               # [VD, SL]
        msk = np.ascontiguousarray(np.broadcast_to(moff, (128, SL)))
        QH = QC * NSB
        in_maps.append(
            {
                "blobA": np.concatenate(
                    [_dev_pack(vG), wv[0]], axis=1
                ).astype(bf16),
                "blobB": np.concatenate(
                    [qT[:, 0:QH], wq[0]], axis=1
                ).astype(bf16),
                "blobC": np.concatenate(
                    [qT[:, QH : 2 * QH], wq[1], wv[1]], axis=1
                ).astype(bf16),
                "msk": msk,
                **shared,
            }
        )
    return in_maps


def kernel(v, q, box_mask, tags_attention, Wv, bv, Wq, bq, Wl, bl):
    # bl shifts all unmasked logits uniformly -> cancels in softmax.
    # tags_attention is unused by the reference module.
    plan = _plan(box_mask)
    batA, batB, kidx, K0, K1 = plan
    KMAX = max(K0, K1)
    key = (K0, K1)
    if key not in _CACHE:
        _CACHE[key] = _build_nc(K0, K1)
    nc = _CACHE[key]
    in_maps = _prep_in_maps(v, q, box_mask, Wv, bv, Wq, bq, Wl, plan)
    res = bass_utils.run_bass_kernel_spmd(
        nc,
        in_maps,
        core_ids=list(range(NCORES)),
        trace=bool(os.environ.get("KERNEL_TRACE")),
        tmpdir=os.environ.get("KERNEL_TMPDIR"),
    )
    _CACHE["last_result"] = res
    out = np.zeros((B, N * S, K), np.float32)
    for c in range(NCORES):
        w = (
            res.results[c]["out"]
            .reshape(128, NS // 128, KMAX)
            .transpose(1, 0, 2)
            .reshape(NS, KMAX)
        )
        for bi, off in [(batA[c], 0), (batB[c], NSB)]:
            ks = kidx[bi]
            if len(ks) == 0:
                out[bi, :, :] = 1.0 / K
            else:
                out[bi, :, ks] = w[off : off + NSB, : len(ks)].T
    return out.reshape(B, N, S, K)
